# revision 3
# baseline (speedup 1.0000x reference)
"""Trainium2 Bass kernel for nn_Block_90726889161490 (sparse_attention).

Reference computation (B=4, T=2048, HIDDEN=1024, 16 heads x 64):
    LayerNorm -> fused qkvp projection (7*HIDDEN cols) -> identity seq
    "compression" (scale 1.0) -> rotary(q, k) -> full softmax attention ->
    GELU side branch on p -> concat([o, p]) @ w_out + b_out.

Sharding: 8 cores = 4 batches x 2 head-groups (tensor parallel over heads
for q/k/v/attention, column split of in_proj, row split of out_proj).
Each core computes a partial [T, HIDDEN] output; host sums the two
head-group partials per batch (the all-reduce after out_proj).

Per-core pipeline (T=2048 tokens, 8 heads):
  A:  LN (bn_stats) + PE transpose -> xnT [128, 8, 2048] bf16
  B1: q/k/v projection (fp32 psum chains, bf16 operands), fused rotary on
      the psum eviction, PE transpose of q/k -> qT/kT [128, 4, 2048] bf16,
      v -> v_aug [128, 16, 8, 65] bf16 (65th col = ones for the softmax
      denominator)
  B2: p projection in [pcol, tok] layout + exact GELU + out_proj p-part
      (+ bias) -> out_acc DRAM  (interleaved with C so PE fills ACT stalls)
  C:  per (i-chunk, head): S^T = kT.T @ qT singles, exp(0.125*S) -> bf16,
      A^T V via psum accumulation chain, softmax denominator from the ones
      column, normalize -> oT [128, 4, 2048] bf16
  D:  out_proj o-part + out_acc + final store.
"""

import os
import sys

for _p in ("/opt/trn_rl_repo", "/root/.axon_site/_ro/trn_rl_repo"):
    if os.path.isdir(_p) and _p not in sys.path:
        sys.path.insert(0, _p)

import numpy as np
import ml_dtypes

import concourse.bass as bass
import concourse.mybir as mybir
import concourse.tile as tile
from concourse import bacc
from concourse.bass_utils import run_bass_kernel_spmd
from concourse.masks import make_identity

F32 = mybir.dt.float32
BF16 = mybir.dt.bfloat16
AF = mybir.ActivationFunctionType
ALU = mybir.AluOpType

N_CORES = 8
B, T, HIDDEN = 4, 2048, 1024
HEADS, HEAD_DIM = 16, 64
HG = HEADS // 2          # heads per core = 8
QK = HG * HEAD_DIM       # q/k/v col-slice per core = 512
PCOLS = 4 * HIDDEN // 2  # p col-slice per core = 2048
KO = HIDDEN // 128       # 8 contraction subtiles for d=1024
TT = T // 128            # 16 token tiles
IC = T // 512            # 4 attention i-chunks
JC = T // 128            # 16 attention j-chunks
LN_EPS = 1e-5


def _build_nc():
    nc = bacc.Bacc("TRN2", target_bir_lowering=False, debug=False)

    x = nc.dram_tensor("x", [T, HIDDEN], F32, kind="ExternalInput")
    gamma = nc.dram_tensor("gamma", [HIDDEN], F32, kind="ExternalInput")
    beta = nc.dram_tensor("beta", [HIDDEN], F32, kind="ExternalInput")
    w_qkv = nc.dram_tensor("w_qkv", [128, KO, 3 * QK], BF16, kind="ExternalInput")
    w_p = nc.dram_tensor("w_p", [128, KO, PCOLS], BF16, kind="ExternalInput")
    w_oo = nc.dram_tensor("w_oo", [128, 4, HIDDEN], BF16, kind="ExternalInput")
    w_op = nc.dram_tensor("w_op", [128, 16, HIDDEN], BF16, kind="ExternalInput")
    bvec = nc.dram_tensor("bvec", [HIDDEN], F32, kind="ExternalInput")
    cos_t = nc.dram_tensor("cos_t", [T, 32], F32, kind="ExternalInput")
    sin_t = nc.dram_tensor("sin_t", [T, 32], F32, kind="ExternalInput")
    out = nc.dram_tensor("out", [T, HIDDEN], F32, kind="ExternalOutput")
    out_acc = nc.dram_tensor("out_acc", [T, HIDDEN], F32)

    def bcast_ap(vec_ap, parts=128):
        return bass.AP(tensor=vec_ap.tensor, offset=vec_ap.offset,
                       ap=[[0, parts]] + list(vec_ap.ap))

    with tile.TileContext(nc) as tc:
        # ---- long-lived tensors ------------------------------------------
        persist_cm = tc.tile_pool(name="persist", bufs=1)
        persist = persist_cm.__enter__()
        xnT = persist.tile([128, KO, T], BF16)        # 32 KB/part
        qT = persist.tile([128, 4, T], BF16)          # 16
        kT = persist.tile([128, 4, T], BF16)          # 16
        v_aug = persist.tile([128, JC, HG, 65], BF16)  # 16.3
        oT = persist.tile([128, 4, T], BF16)          # 16
        w_op_sb = persist.tile([128, 16, HIDDEN], BF16)  # 32
        w_oo_sb = persist.tile([128, 4, HIDDEN], BF16)   # 8
        bvec_sb = persist.tile([128, HIDDEN], F32)       # 4
        ident = persist.tile([128, 128], F32)
        nc.sync.dma_start(w_op_sb[:], w_op[:])
        nc.sync.dma_start(w_oo_sb[:], w_oo[:])
        nc.gpsimd.dma_start(out=bvec_sb[:], in_=bcast_ap(bvec.ap()))
        make_identity(nc, ident)
        nc.vector.memset(v_aug[:, :, :, 64], 1.0)

        # ---- stage A: LayerNorm + transpose to xnT -----------------------
        with tc.tile_pool(name="ln", bufs=3) as ln_pool, \
             tc.tile_pool(name="ln1", bufs=1) as ln1, \
             tc.tile_pool(name="ln_ps", bufs=4, space="PSUM") as ln_ps:
            gamma_sb = ln1.tile([128, HIDDEN], F32)
            beta_sb = ln1.tile([128, HIDDEN], F32)
            eps_sb = ln1.tile([128, 1], F32)
            nc.gpsimd.dma_start(out=gamma_sb[:], in_=bcast_ap(gamma.ap()))
            nc.gpsimd.dma_start(out=beta_sb[:], in_=bcast_ap(beta.ap()))
            nc.vector.memset(eps_sb[:], LN_EPS)
            for tt in range(TT):
                xt = ln_pool.tile([128, HIDDEN], F32, tag="xt")
                nc.sync.dma_start(xt[:], x[tt * 128:(tt + 1) * 128, :])
                stats = ln_pool.tile([128, 2, 6], F32, tag="st")
                xr = xt[:].rearrange("p (s d) -> p s d", s=2)
                for i in range(2):
                    nc.vector.bn_stats(out=stats[:, i, :], in_=xr[:, i, :])
                mv = ln_pool.tile([128, 2], F32, tag="mv")
                nc.vector.bn_aggr(out=mv[:], in_=stats[:])
                std = ln_pool.tile([128, 1], F32, tag="sd")
                nc.scalar.activation(out=std[:], in_=mv[:, 1:2], func=AF.Sqrt,
                                     bias=eps_sb[:])
                rstd = ln_pool.tile([128, 1], F32, tag="rs")
                nc.vector.reciprocal(out=rstd[:], in_=std[:])
                nc.vector.tensor_scalar(out=xt[:], in0=xt[:], scalar1=mv[:, 0:1],
                                        scalar2=rstd[:], op0=ALU.subtract,
                                        op1=ALU.mult)
                nc.gpsimd.tensor_mul(xt[:], xt[:], gamma_sb[:])
                nc.gpsimd.tensor_add(xt[:], xt[:], beta_sb[:])
                for ks in range(KO):
                    ps = ln_ps.tile([128, 128], F32, tag="tr")
                    nc.tensor.transpose(ps[:], xt[:, ks * 128:(ks + 1) * 128],
                                        ident[:])
                    nc.scalar.copy(out=xnT[:, ks, tt * 128:(tt + 1) * 128],
                                   in_=ps[:])

        # ---- stage B1: q/k/v projection + rotary + transposes ------------
        with tc.tile_pool(name="b1w", bufs=2) as b1w, \
             tc.tile_pool(name="b1t", bufs=3) as b1t, \
             tc.tile_pool(name="b1c", bufs=1) as b1c, \
             tc.tile_pool(name="b1_ps", bufs=2, space="PSUM") as b1_ps, \
             tc.tile_pool(name="b1_ps2", bufs=3, space="PSUM") as b1_ps2:
            cos_sb = b1c.tile([128, TT, 32], F32)
            sin_sb = b1c.tile([128, TT, 32], F32)
            nc.sync.dma_start(cos_sb[:], cos_t.ap().rearrange("(t p) f -> p t f", p=128))
            nc.sync.dma_start(sin_sb[:], sin_t.ap().rearrange("(t p) f -> p t f", p=128))
            for cc in range(3):  # 0: q, 1: k, 2: v
                wt = b1w.tile([128, KO, QK], BF16, tag="w")
                nc.sync.dma_start(wt[:], w_qkv[:, :, cc * QK:(cc + 1) * QK])
                for tt in range(TT):
                    ps = b1_ps.tile([128, QK], F32, tag="mm")
                    for ks in range(KO):
                        nc.tensor.matmul(ps[:], xnT[:, ks, tt * 128:(tt + 1) * 128],
                                         wt[:, ks, :], start=(ks == 0),
                                         stop=(ks == KO - 1))
                    if cc == 2:
                        # v: scatter heads into v_aug [:, tt, h, 0:64]
                        pv = ps[:].rearrange("p (h d) -> p h d", h=HG)
                        nc.vector.tensor_copy(out=v_aug[:, tt, :, 0:64], in_=pv)
                    else:
                        # rotary on psum: view [128, h, 2, 32]
                        pr = ps[:].rearrange("p (h two f) -> p h two f", h=HG, two=2)
                        cosb = cos_sb[:, tt, None, :].to_broadcast((128, HG, 32))
                        sinb = sin_sb[:, tt, None, :].to_broadcast((128, HG, 32))
                        rot = b1t.tile([128, HG, 2, 32], F32, tag="rot")
                        ta = b1t.tile([128, HG, 32], F32, tag="ta")
                        tb = b1t.tile([128, HG, 32], F32, tag="tb")
                        # x1' = x1*cos - x2*sin
                        nc.vector.tensor_mul(ta[:], pr[:, :, 1, :], sinb)
                        nc.vector.tensor_mul(tb[:], pr[:, :, 0, :], cosb)
                        nc.vector.tensor_sub(rot[:, :, 0, :], tb[:], ta[:])
                        # x2' = x1*sin + x2*cos
                        nc.vector.tensor_mul(ta[:], pr[:, :, 0, :], sinb)
                        nc.vector.tensor_mul(tb[:], pr[:, :, 1, :], cosb)
                        nc.vector.tensor_add(rot[:, :, 1, :], tb[:], ta[:])
                        dst = qT if cc == 0 else kT
                        rflat = rot[:].rearrange("p h two f -> p (h two f)")
                        for hc in range(4):
                            ps2 = b1_ps2.tile([128, 128], F32, tag="tr2")
                            nc.tensor.transpose(
                                ps2[:], rflat[:, hc * 128:(hc + 1) * 128], ident[:])
                            nc.scalar.copy(
                                out=dst[:, hc, tt * 128:(tt + 1) * 128], in_=ps2[:])

        # ---- stages C + B2 interleaved -----------------------------------
        with tc.tile_pool(name="c_e", bufs=4) as c_e, \
             tc.tile_pool(name="c_t", bufs=3) as c_t, \
             tc.tile_pool(name="b2_t", bufs=2) as b2_t, \
             tc.tile_pool(name="b2_pt", bufs=1) as b2_pt, \
             tc.tile_pool(name="b2_w", bufs=2) as b2_w, \
             tc.tile_pool(name="c_ps_s", bufs=2, space="PSUM") as c_ps_s, \
             tc.tile_pool(name="c_ps_o", bufs=2, space="PSUM") as c_ps_o, \
             tc.tile_pool(name="b2_ps", bufs=2, space="PSUM") as b2_ps, \
             tc.tile_pool(name="b2_ps_o", bufs=2, space="PSUM") as b2_ps_o:
            for ic in range(IC):
                isl = slice(ic * 512, (ic + 1) * 512)
                # ---- attention for this i-chunk ----
                for h in range(HG):
                    hb = (h % 2) * 64
                    hc = h // 2
                    po = c_ps_o.tile([65, 512], F32, tag="po")
                    for jc in range(JC):
                        s_ps = c_ps_s.tile([128, 512], F32, tag="s")
                        nc.tensor.matmul(
                            s_ps[:],
                            kT[hb:hb + 64, hc, jc * 128:(jc + 1) * 128],
                            qT[hb:hb + 64, hc, isl],
                            start=True, stop=True)
                        e = c_e.tile([128, 512], BF16, tag="e")
                        nc.scalar.activation(e[:], s_ps[:], AF.Exp, scale=0.125)
                        nc.tensor.matmul(po[:], v_aug[:, jc, h, :], e[:],
                                         start=(jc == 0), stop=(jc == JC - 1))
                    z = c_t.tile([1, 512], F32, tag="z")
                    nc.vector.tensor_copy(z[:], po[64:65, :])
                    zb = c_t.tile([64, 512], F32, tag="zb")
                    nc.gpsimd.partition_broadcast(zb[:], z[:])
                    rz = c_t.tile([64, 512], F32, tag="rz")
                    nc.vector.reciprocal_approx_fast(rz[:], zb[:])
                    nc.vector.tensor_mul(oT[hb:hb + 64, hc, isl], po[0:64, :], rz[:])
                # ---- B2: p branch for this i-chunk ----
                pt = b2_pt.tile([128, 16, 512], BF16, tag="pt")
                for pc in range(16):
                    wpt = b2_w.tile([128, KO, 128], BF16, tag="wp")
                    nc.sync.dma_start(wpt[:], w_p[:, :, pc * 128:(pc + 1) * 128])
                    pp = b2_ps.tile([128, 512], F32, tag="pp")
                    for ks in range(KO):
                        nc.tensor.matmul(pp[:], wpt[:, ks, :], xnT[:, ks, isl],
                                         start=(ks == 0), stop=(ks == KO - 1))
                    nc.scalar.activation(pt[:, pc, :], pp[:], AF.Gelu)
                for isub in range(4):
                    tok0 = ic * 512 + isub * 128
                    for oc in range(2):
                        po2 = b2_ps_o.tile([128, 512], F32, tag="po2")
                        for pc in range(16):
                            nc.tensor.matmul(
                                po2[:],
                                pt[:, pc, isub * 128:(isub + 1) * 128],
                                w_op_sb[:, pc, oc * 512:(oc + 1) * 512],
                                start=(pc == 0), stop=(pc == 15))
                        acc = b2_t.tile([128, 512], F32, tag="acc")
                        nc.vector.tensor_add(acc[:], po2[:],
                                             bvec_sb[:, oc * 512:(oc + 1) * 512])
                        nc.sync.dma_start(
                            out_acc[tok0:tok0 + 128, oc * 512:(oc + 1) * 512],
                            acc[:])

        # ---- stage D: out_proj o-part + accumulate -----------------------
        with tc.tile_pool(name="d_t", bufs=3) as d_t, \
             tc.tile_pool(name="d_ps", bufs=3, space="PSUM") as d_ps:
            for tt in range(TT):
                accs = d_t.tile([128, HIDDEN], F32, tag="dacc")
                nc.sync.dma_start(accs[:], out_acc[tt * 128:(tt + 1) * 128, :])
                fin = d_t.tile([128, HIDDEN], F32, tag="fin")
                for oc in range(2):
                    po3 = d_ps.tile([128, 512], F32, tag="po3")
                    for ks in range(4):
                        nc.tensor.matmul(
                            po3[:],
                            oT[:, ks, tt * 128:(tt + 1) * 128],
                            w_oo_sb[:, ks, oc * 512:(oc + 1) * 512],
                            start=(ks == 0), stop=(ks == 3))
                    nc.vector.tensor_add(fin[:, oc * 512:(oc + 1) * 512], po3[:],
                                         accs[:, oc * 512:(oc + 1) * 512])
                nc.sync.dma_start(out[tt * 128:(tt + 1) * 128, :], fin[:])

        persist_cm.__exit__(None, None, None)

    nc.compile()
    return nc


_NC_CACHE = None


def _get_nc():
    global _NC_CACHE
    if _NC_CACHE is None:
        _NC_CACHE = _build_nc()
    return _NC_CACHE


def _host_tables():
    inv_freq = 1.0 / (10000.0 ** (np.arange(0, HEAD_DIM, 2, dtype=np.float32)
                                  / HEAD_DIM))
    ang = np.arange(T, dtype=np.float32)[:, None] * inv_freq[None, :]
    return np.cos(ang).astype(np.float32), np.sin(ang).astype(np.float32)


def _shard_weights(w_in, w_out, b_out, ln_gamma, ln_beta, x):
    cos_np, sin_np = _host_tables()
    bf = ml_dtypes.bfloat16

    def fold(a, ko):
        # [ko*128, c] -> [128, ko, c] with [p, k, c] = a[k*128 + p, c]
        return np.ascontiguousarray(
            a.reshape(ko, 128, a.shape[1]).transpose(1, 0, 2))

    in_maps = []
    for c in range(N_CORES):
        b, g = c // 2, c % 2
        sl = slice(g * QK, (g + 1) * QK)
        w_qkv = np.concatenate(
            [w_in[:, 0 * HIDDEN:][:, sl], w_in[:, 1 * HIDDEN:][:, sl],
             w_in[:, 2 * HIDDEN:][:, sl]], axis=1)
        w_p = w_in[:, 3 * HIDDEN + g * PCOLS:3 * HIDDEN + (g + 1) * PCOLS]
        w_oo = w_out[g * QK:(g + 1) * QK, :]
        w_op = w_out[HIDDEN + g * PCOLS:HIDDEN + (g + 1) * PCOLS, :]
        in_maps.append({
            "x": np.ascontiguousarray(x[b]).astype(np.float32),
            "gamma": ln_gamma.astype(np.float32),
            "beta": ln_beta.astype(np.float32),
            "w_qkv": fold(w_qkv, KO).astype(bf),
            "w_p": fold(w_p, KO).astype(bf),
            "w_oo": fold(w_oo, 4).astype(bf),
            "w_op": fold(w_op, 16).astype(bf),
            "bvec": (b_out if g == 0 else np.zeros_like(b_out)).astype(np.float32),
            "cos_t": cos_np,
            "sin_t": sin_np,
        })
    return in_maps


def kernel(x, ln_gamma, ln_beta, w_in, w_out, b_out, _trace=False, _tmpdir=None):
    x = np.asarray(x, dtype=np.float32)
    ln_gamma = np.asarray(ln_gamma, dtype=np.float32)
    ln_beta = np.asarray(ln_beta, dtype=np.float32)
    w_in = np.asarray(w_in, dtype=np.float32)
    w_out = np.asarray(w_out, dtype=np.float32)
    b_out = np.asarray(b_out, dtype=np.float32)

    nc = _get_nc()
    in_maps = _shard_weights(w_in, w_out, b_out, ln_gamma, ln_beta, x)
    kwargs = {}
    if _trace:
        kwargs = {"trace": True, "tmpdir": _tmpdir}
    res = run_bass_kernel_spmd(nc, in_maps, core_ids=list(range(N_CORES)),
                               **kwargs)
    outs = [res.results[c]["out"] for c in range(N_CORES)]
    full = np.stack([outs[2 * b] + outs[2 * b + 1] for b in range(B)], axis=0)
    kernel._last_exec_time_ns = res.exec_time_ns
    return full.astype(np.float32)
